# revision 14
# baseline (speedup 1.0000x reference)
"""Trainium2 Bass kernel for nn_AffineAttentionNN (moe_routing).

Math (per the reference):
    dist_sq[n,c] = ||x[n]-ctrs[c]||^2_s   (s-weighted squared distance)
    a = softmax(-dist_sq, axis=c)
    out = einsum('nc,ng,cgp->np', a, x, Wv) + a @ Ov

Device decomposition (data-parallel over n across 8 cores; per core n_loc=2048):
  - Softmax offsets: the per-row term (x*x)@s is constant along c and cancels;
    we exponentiate g[c,n] = 2(x*s)@ctrs.T - ccs[c] directly.
  - All heavy tensors in bf16: the per-expert value matmul runs 1 cyc/col on
    PE, the routing-weight multiply runs 2 elem/cyc/lane on DVE (2x_1P mode),
    and the partition-broadcast DMA volume halves vs fp32.
  - Per expert c: e_c must appear on all 128 partitions to scale x. Supply it
    from three sources, balanced so no engine saturates:
      * partition-broadcast DMA of PAIRS of adjacent e-rows (1 MB transfers)
        on BOTH hardware DGE rings (SP/sync and Activation/scalar — each
        sustains ~200 GB/s independently);
      * PE one-hot matmul replication (identity column c) into PSUM,
        evacuated to SBUF bf16 by ScalarE — only for the front pairs (hides
        the e->DRAM->broadcast latency at start) and a few tail pairs;
    The elementwise multiply z = x * er runs on the DVE (2 elem/cyc/lane
    bf16) for most pairs and on the otherwise-idle GPSIMD for a slice of
    mid-kernel pairs, keeping the DVE off the critical path.
      gT = matmul(lhsT=2*s*ctrs (g,c), rhs=xT) (f32r)        [c, n] PSUM
      eT = Exp(gT + bias=-ccs[c])  on ScalarE -> bf16        [c, n]
      Z  = matmul(lhsT=ones, rhs=eT) -> reciprocal           [1, n]
      per expert c: er = broadcast(e[c,:]) via DMA or PE+ACT [g, n] bf16
                    z  = xT_bf * er   (VectorE/GPSIMD, bf16) [g, n]
                    outT[p, js] += matmul(Wv[c] (g,p), z)    PSUM acc
      outT += matmul(lhsT=Ov (c,p), rhs=eT)                  (Ov term)
      out  = outT * recipZ_rep  (normalize), DMA out, host transposes.
"""

import os
import numpy as np
from contextlib import ExitStack

import concourse.bass as bass
import concourse.tile as tile
from concourse import mybir

N, D, C, P = 16384, 128, 128, 128
N_CORES = 8
N_LOC = N // N_CORES          # 2048
CHUNK = 512                   # PSUM bank width (fp32)
NCH = N_LOC // CHUNK          # 4

F32 = mybir.dt.float32
F32R = mybir.dt.float32r
BF16 = mybir.dt.bfloat16

N_PAIRS = C // 2
# Leading pairs on the PE-replication path: their supply comes from eT in
# SBUF, hiding the e->DRAM->broadcast latency at kernel start.
FRONT_PE = int(os.environ.get("KERNEL_FRONT_PE", "5"))
# Trailing pairs on the PE path so the end of the broadcast stream can't
# starve the consumer.
TAIL_PE = int(os.environ.get("KERNEL_TAIL_PE", "2"))
# Extra PE-replicated pairs spread through the middle.
MID_PE = int(os.environ.get("KERNEL_MID_PE", "0"))
PE_PAIRS = FRONT_PE + TAIL_PE + MID_PE
# How many experts ahead the er production runs of the consuming multiply.
LOOKAHEAD = int(os.environ.get("KERNEL_LOOKAHEAD", "16"))
# Pairs whose routing-weight multiply runs on GPSIMD instead of the DVE.
# DEAD on TRN2: GPSIMD and DVE share their SBUF read/write ports, so
# concurrent tensor ops on the two engines serialize (measured: DVE pair
# multiplies overlapping GPSIMD ones balloon 2.3us -> 10us). Keep 0.
GP_PAIRS = int(os.environ.get("KERNEL_GP_PAIRS", "0"))
GP_FIRST = int(os.environ.get("KERNEL_GP_FIRST", "6"))
GP_LAST = int(os.environ.get("KERNEL_GP_LAST", "56"))
# Evacuate the FRONT pairs' first-half PSUM tiles on the (ramp-idle) DVE and
# the second halves on ScalarE, halving the startup load on each engine.
FRONT_DVE = os.environ.get("KERNEL_FRONT_DVE", "1") == "1"
# PE warmup: dummy matmuls at t=0 (no input deps) ramp the tensor engine's
# p-state while the prologue DMAs land, so the distance matmuls and the first
# value matmuls run at full clock instead of 0.65 GHz.
WARMUP_MM = int(os.environ.get("KERNEL_WARMUP_MM", "10"))


def _pair_is_pe(p):
    """FRONT_PE leading + TAIL_PE trailing pairs, Bresenham spread between."""
    if p < FRONT_PE or p >= N_PAIRS - TAIL_PE:
        return True
    rest = MID_PE
    span = N_PAIRS - FRONT_PE - TAIL_PE
    q = p - FRONT_PE
    return ((q + 1) * rest) // span - (q * rest) // span == 1


_BC_PAIRS = [p for p in range(N_PAIRS) if not _pair_is_pe(p)]


def _pair_is_gp(p):
    """GP_PAIRS broadcast-supplied pairs spread over [GP_FIRST, GP_LAST]."""
    if _pair_is_pe(p) or not (GP_FIRST <= p <= GP_LAST) or GP_PAIRS <= 0:
        return False
    window = [q for q in _BC_PAIRS if GP_FIRST <= q <= GP_LAST]
    k = window.index(p)
    m = len(window)
    return (k * GP_PAIRS) // m != ((k + 1) * GP_PAIRS) // m


def _dedup_ldweights(nc):
    """Each matmul is emitted as an InstLdweights + non-self-loading
    InstMatmult pair; a run of matmuls sharing the same stationary operand
    re-loads it before every matmul, which blocks fill/drain overlap between
    them (379 ns/MM instead of ~216). Delete the redundant loads. The
    schedule is final here, so block order IS the PE execution order; the
    deleted loads carry no sync_info and no one references them. bf16-only
    out of caution (f32r has a known walrus quirk around non-self-loading)."""
    n = 0
    for f in nc.m.functions:
        for blk in f.blocks:
            last_sig = None
            keep = []
            for inst in blk.instructions:
                if str(inst.engine) != "EngineType.PE":
                    keep.append(inst)
                    continue
                if isinstance(inst, mybir.InstLdweights):
                    w = inst.ins[0]
                    si = inst.sync_info
                    clean = si is None or (not si.on_wait and not si.on_update)
                    if (w.dtype == mybir.dt.bfloat16 and clean
                            and inst.perf_mode is None
                            and inst.is_transpose is None):
                        sig = (str(w.ap), w.offset, str(w.memref))
                        if sig == last_sig:
                            n += 1
                            continue  # drop the redundant load
                        last_sig = sig
                    else:
                        last_sig = None
                elif isinstance(inst, mybir.InstMatmult):
                    if inst.ldweights is not False or inst.is_transpose:
                        last_sig = None  # self-loading matmul replaces weights
                elif isinstance(inst, (mybir.InstEventSemaphore, mybir.InstDrain)):
                    pass  # no effect on the loaded weights
                else:
                    last_sig = None
                keep.append(inst)
            blk.instructions = keep
    return n


def _legalize_waits(nc, max_waits=1):
    """This walrus build accepts at most one sync-wait per instruction; Tile
    emits several. Hoist the excess onto standalone single-wait
    InstEventSemaphore ops just before the owner on the same engine stream."""
    import bass_rust

    n = 0
    for f in nc.m.functions:
        for blk in f.blocks:
            out = []
            for inst in blk.instructions:
                si = getattr(inst, "sync_info", None)
                waits = list(si.on_wait) if si is not None else []
                if len(waits) > max_waits:
                    extra, keep = waits[:-max_waits], waits[-max_waits:]
                    for w in extra:
                        n += 1
                        ev = mybir.InstEventSemaphore(
                            name=f"legal_wait_{n}_{inst.name}", ins=[], outs=[]
                        )
                        ev.engine = inst.engine
                        ev.sync_info = bass_rust.SyncInfo(on_wait=[w], on_update=[])
                        out.append(ev)
                    inst.sync_info = bass_rust.SyncInfo(
                        on_wait=keep, on_update=list(si.on_update)
                    )
                out.append(inst)
            blk.instructions = out
    return n


def _emit_kernel(tc, aps):
    nc = tc.nc
    xT, wvT, bfpack, c2sT, nccs, outT = (
        aps["xT"], aps["wvT"], aps["bfpack"], aps["c2sT"],
        aps["nccs"], aps["outT"],
    )

    with ExitStack() as ctx:
        const = ctx.enter_context(tc.tile_pool(name="const", bufs=1))
        dram = ctx.enter_context(tc.tile_pool(name="dram", bufs=1, space="DRAM"))
        erep_p = ctx.enter_context(tc.tile_pool(name="erep", bufs=9))
        erpe_p = ctx.enter_context(tc.tile_pool(name="erpe", bufs=3))
        z_p = ctx.enter_context(tc.tile_pool(name="zt", bufs=4))
        out_p = ctx.enter_context(tc.tile_pool(name="outs", bufs=1))

        # ---- PE warmup: no-dep dummy matmuls ramp the clock during the
        # prologue DMAs ----
        ones_s = const.tile([C, 1], BF16, tag="ones")
        nc.vector.memset(ones_s[:], 1.0)
        if WARMUP_MM:
            warm_in = const.tile([C, CHUNK], BF16, tag="warm_in")
            nc.vector.memset(warm_in[:], 0.0)
            with tc.tile_pool(name="psum_w", bufs=1, space="PSUM") as psum_w:
                pw = psum_w.tile([C, CHUNK], F32, tag="pw")
                sel_w = ones_s[:, 0:1].broadcast_to([C, C])
                for _ in range(WARMUP_MM):
                    nc.tensor.matmul(pw[:], sel_w, warm_in[:],
                                     start=True, stop=True)

        # ---- constants / inputs into SBUF ----
        # The prologue (distance matmul -> exp) gates the whole pipeline, so
        # its inputs load first on the sync ring, fewest triggers first: the
        # bf16 constants (id | ov | xTb) ride as ONE host-packed DMA. Wv and
        # the rz chain ride the SWDGE ring (GPSIMD is otherwise idle and both
        # HWDGE rings are saturated with e-broadcasts).
        c2s_s = const.tile([D, C], F32R, tag="c2s")
        nc.sync.dma_start(c2s_s[:], c2sT[:, :])
        nccs_s = const.tile([C, 1], F32, tag="nccs")
        nc.sync.dma_start(nccs_s[:], nccs[:, :])
        # prewarm the exp table off the first const load so ACT_TABLE_LOAD
        # isn't serialized into the first real activation's dependency chain
        warm_s = const.tile([C, 1], F32, tag="warm")
        nc.scalar.activation(
            warm_s[:], nccs_s[:], mybir.ActivationFunctionType.Exp)
        xT_s = const.tile([D, N_LOC], F32R, tag="xT")
        nc.sync.dma_start(xT_s[:, 0:CHUNK], xT[:, 0:CHUNK])
        bfp_s = const.tile([C, C + P + N_LOC], BF16, tag="bfp")
        nc.sync.dma_start(bfp_s[:], bfpack[:, :])
        id_s = bfp_s[:, 0:C]
        ov_s = bfp_s[:, C:C + P]
        xTb_s = bfp_s[:, C + P:]
        for j in range(1, NCH):
            js = slice(j * CHUNK, (j + 1) * CHUNK)
            nc.sync.dma_start(xT_s[:, js], xT[:, js])
        WVCH = C * P // 8
        wv_s = const.tile([D, C * P], BF16, tag="wv")
        for k in range(8):
            nc.gpsimd.dma_start(
                wv_s[:, k * WVCH:(k + 1) * WVCH], wvT[:, k * WVCH:(k + 1) * WVCH])
        eT_s = const.tile([C, N_LOC], BF16, tag="eT")
        rz_s = const.tile([1, N_LOC], F32, tag="rz")
        zc_s = const.tile([P, N_LOC // P], F32, tag="zc")
        zcr_s = const.tile([P, N_LOC // P], F32, tag="zcr")
        rzrep_s = const.tile([P, N_LOC], F32, tag="rzrep")

        e_dram = dram.tile([N_PAIRS, 2 * N_LOC], BF16, tag="e_dram")
        rz_dram = dram.tile([1, N_LOC], F32, tag="rz_dram")
        rzr_dram = dram.tile([1, N_LOC], F32, tag="rzr_dram")

        # ---- prologue: distances -> unnormalized softmax weights eT [c, n] ----
        with ExitStack() as dctx:
            psum_d = dctx.enter_context(
                tc.tile_pool(name="psum_d", bufs=2, space="PSUM"))
            psum_z = dctx.enter_context(
                tc.tile_pool(name="psum_z", bufs=1, space="PSUM"))
            for j in range(NCH):
                js = slice(j * CHUNK, (j + 1) * CHUNK)
                pd = psum_d.tile([C, CHUNK], F32, tag="pd")
                nc.tensor.matmul(pd[:], c2s_s[:], xT_s[:, js], start=True, stop=True)
                nc.scalar.activation(
                    eT_s[:, js], pd[:], mybir.ActivationFunctionType.Exp,
                    bias=nccs_s[:, 0:1], scale=1.0,
                )
                pz = psum_z.tile([1, CHUNK], F32, tag="pz")
                nc.tensor.matmul(pz[:], ones_s[:], eT_s[:, js], start=True, stop=True)
                nc.scalar.copy(rz_s[0:1, js], pz[0:1, :])

        # e -> DRAM (paired-row layout) for the partition-broadcast DMAs;
        # 1/Z on all 128 lanes via a strided-DMA transpose roundtrip. The rz
        # chain rides the SWDGE ring: its writes wait on compute, and on the
        # sync FIFO they would block every pair-broadcast queued behind them.
        e_flat = e_dram[:, :].rearrange("a (b n) -> (a b) n", b=2)
        nc.sync.dma_start(e_flat, eT_s[:])
        nc.gpsimd.dma_start(rz_dram[:, :], rz_s[:])
        nc.gpsimd.dma_start(
            zc_s[:], rz_dram[0:1, :].rearrange("o (f p) -> (o p) f", p=P))
        nc.vector.reciprocal(zcr_s[:], zc_s[:])
        nc.gpsimd.dma_start(
            rzr_dram[0:1, :].rearrange("o (f p) -> (o p) f", p=P), zcr_s[:])
        nc.gpsimd.dma_start(rzrep_s[:], rzr_dram[0:1, :].partition_broadcast(P))

        # ---- main expert loop, accumulate outT in PSUM ----
        psum_o = ctx.enter_context(tc.tile_pool(name="psum_o", bufs=1, space="PSUM"))
        psum_r = ctx.enter_context(tc.tile_pool(name="psum_r", bufs=2, space="PSUM"))
        po = psum_o.tile([P, N_LOC], F32, tag="po")

        # Ov term opens each chunk's accumulation group (it only needs eT), so
        # the tail chain is just last-expert-matmul -> normalize -> store.
        for j in range(NCH):
            js = slice(j * CHUNK, (j + 1) * CHUNK)
            nc.tensor.matmul(
                po[:, js], ov_s[:], eT_s[:, js],
                start=True, stop=False, skip_group_check=True,
            )

        er_pair = {}   # pair -> er2 tile [D, 2*N_LOC]
        REP_HALF = N_LOC // 2
        # x repeated twice along a stride-0 dim, for one multiply per pair
        x_rep2 = xTb_s[:].unsqueeze(1).broadcast_to([D, 2, N_LOC])

        def emit_replicate(c):
            """PE one-hot matmul: replicate eT row c to all partitions, in two
            [D, N_LOC/2] PSUM tiles, each evacuated to SBUF bf16. Front pairs
            may evacuate on the (ramp-idle) DVE for a faster startup cadence;
            the rest evacuate on ScalarE. The two experts of a pair share one
            [D, 2*N_LOC] SBUF tile."""
            p, half = divmod(c, 2)
            if half == 0:
                erpe = erpe_p.tile([D, 2 * N_LOC], BF16, tag="erpe")
                er_pair[p] = erpe
            er = er_pair[p]
            # one-hot weights: identity column c repeated via a stride-0 dim
            sel_c = id_s[:, c:c + 1].broadcast_to([C, P])
            for h in range(2):
                dst = slice(half * N_LOC + h * REP_HALF,
                            half * N_LOC + (h + 1) * REP_HALF)
                pr = psum_r.tile([D, REP_HALF], F32, tag="pr")
                for q in range(REP_HALF // CHUNK):
                    qs = slice(q * CHUNK, (q + 1) * CHUNK)
                    src = slice(h * REP_HALF + q * CHUNK,
                                h * REP_HALF + (q + 1) * CHUNK)
                    nc.tensor.matmul(
                        pr[:, qs], sel_c, eT_s[:, src], start=True, stop=True)
                if FRONT_DVE and p < FRONT_PE and h == 0:
                    nc.vector.tensor_copy(er[:, dst], pr[:])
                else:
                    nc.scalar.copy(er[:, dst], pr[:])

        bc_count = [0]

        def emit_broadcast_pair(p):
            """One 1 MB DMA: broadcast e-rows (2p, 2p+1) to 128 partitions.
            Alternate between the two hardware DGE rings (sync / scalar) —
            each sustains ~200 GB/s independently. SWDGE would serialize
            behind GPSIMD tensor ops."""
            er2 = erep_p.tile([D, 2 * N_LOC], BF16, tag="er2")
            ring = nc.sync if bc_count[0] % 2 == 0 else nc.scalar
            bc_count[0] += 1
            ring.dma_start(er2[:], e_dram[p:p + 1, :].partition_broadcast(D))
            er_pair[p] = er2

        # Producer schedule: front pairs, then the tail pairs (produced early
        # into their own pool so the end of the broadcast stream can't starve
        # the consumer), then everything else in pair order.
        sched = []
        for p in range(N_PAIRS):
            if _pair_is_pe(p):
                sched.append(("rep", 2 * p, True))
                sched.append(("rep", 2 * p + 1, True))
            else:
                sched.append(("bc", p, True))
        prod_i = 0

        def pump(c_needed):
            nonlocal prod_i
            covered = -1
            while prod_i < len(sched) and covered < min(c_needed + LOOKAHEAD, C - 1):
                kind, v, counts = sched[prod_i]
                if kind == "bc":
                    emit_broadcast_pair(v)
                    covered = 2 * v + 1
                else:
                    emit_replicate(v)
                    if counts:
                        covered = v if v % 2 == 1 else v - 1
                prod_i += 1

        for p in range(N_PAIRS):
            pump(2 * p + 1)
            er2 = er_pair.pop(p)
            z2 = z_p.tile([D, 2 * N_LOC], BF16, tag="z2")
            if p < FRONT_PE:
                # startup: one multiply per expert so the first matmuls don't
                # wait for the whole pair's replication
                nc.vector.tensor_mul(z2[:, 0:N_LOC], xTb_s[:], er2[:, 0:N_LOC])
                nc.vector.tensor_mul(z2[:, N_LOC:], xTb_s[:], er2[:, N_LOC:])
            elif _pair_is_gp(p):
                nc.gpsimd.tensor_mul(z2[:], x_rep2, er2[:])
            else:
                nc.vector.tensor_mul(z2[:], x_rep2, er2[:])
            for half in range(2):
                c = 2 * p + half
                wv_c = wv_s[:, c * P:(c + 1) * P]
                for j in range(NCH):
                    js = slice(half * N_LOC + j * CHUNK,
                               half * N_LOC + (j + 1) * CHUNK)
                    ps = slice(j * CHUNK, (j + 1) * CHUNK)
                    nc.tensor.matmul(
                        po[:, ps], wv_c, z2[:, js],
                        start=False, stop=(c == C - 1), skip_group_check=True,
                    )

        # ---- normalize and store per chunk; chunk j only waits on the last
        # expert's chunk-j matmul, so the tail pipelines across PE/DVE/DMA ----
        out_s = out_p.tile([P, N_LOC], F32, tag="out")
        for j in range(NCH):
            js = slice(j * CHUNK, (j + 1) * CHUNK)
            nc.vector.tensor_mul(out_s[:, js], po[:, js], rzrep_s[:, js])
            nc.sync.dma_start(outT[:, js], out_s[:, js])


def build_nc():
    nc = bass.Bass(target_bir_lowering=False, trn_type="TRN2")
    aps = {
        "xT": nc.dram_tensor("xT", [D, N_LOC], F32R, kind="ExternalInput").ap(),
        "wvT": nc.dram_tensor("wvT", [D, C * P], BF16, kind="ExternalInput").ap(),
        "bfpack": nc.dram_tensor(
            "bfpack", [C, C + P + N_LOC], BF16, kind="ExternalInput").ap(),
        "c2sT": nc.dram_tensor("c2sT", [D, C], F32R, kind="ExternalInput").ap(),
        "nccs": nc.dram_tensor("nccs", [C, 1], F32, kind="ExternalInput").ap(),
        "outT": nc.dram_tensor("outT", [P, N_LOC], F32, kind="ExternalOutput").ap(),
    }
    with tile.TileContext(nc) as tc:
        _emit_kernel(tc, aps)
    _dedup_ldweights(nc)
    _legalize_waits(nc)
    return nc


_CACHE = {}


def _get_nc():
    key = (PE_PAIRS, LOOKAHEAD, GP_PAIRS, WARMUP_MM)
    if key not in _CACHE:
        _CACHE[key] = build_nc()
    return _CACHE[key]


def kernel(x, ctrs, Wv, Ov, s, _spmd_kwargs=None):
    import ml_dtypes
    from concourse.bass_utils import run_bass_kernel_spmd

    bf16 = ml_dtypes.bfloat16
    x = np.ascontiguousarray(x, dtype=np.float32)
    ctrs = np.asarray(ctrs, dtype=np.float32)
    Wv = np.asarray(Wv, dtype=np.float32)
    Ov = np.ascontiguousarray(Ov, dtype=np.float32)
    s = np.asarray(s, dtype=np.float32)

    # host-side prep of the small routing constants
    c2sT = np.ascontiguousarray((2.0 * ctrs * s[None, :]).T)        # [g, c]
    nccs = np.ascontiguousarray(-((ctrs * ctrs) @ s)[:, None])      # [c, 1]
    wvT = np.ascontiguousarray(
        Wv.transpose(1, 0, 2).reshape(D, C * P).astype(bf16))       # [g, c*p]
    ov_b = np.ascontiguousarray(Ov.astype(bf16))
    selT = np.eye(C, dtype=np.float32).astype(bf16)

    in_maps = []
    for i in range(N_CORES):
        xi = x[i * N_LOC:(i + 1) * N_LOC]
        xiT = np.ascontiguousarray(xi.T)
        # one bf16 constants DMA: [ id | Ov | x^T ] along the free dim
        bfpack = np.ascontiguousarray(
            np.concatenate([selT, ov_b, xiT.astype(bf16)], axis=1))
        m = {
            "xT": xiT,
            "wvT": wvT,
            "bfpack": bfpack,
            "c2sT": c2sT,
            "nccs": nccs,
        }
        in_maps.append(m)

    nc = _get_nc()
    for attempt in range(3):
        res = run_bass_kernel_spmd(
            nc, in_maps, core_ids=list(range(N_CORES)), **(_spmd_kwargs or {})
        )
        out = np.empty((N, P), dtype=np.float32)
        for i in range(N_CORES):
            out[i * N_LOC:(i + 1) * N_LOC] = res.results[i]["outT"].T
        kernel.last_result = res
        if np.isfinite(out).all():
            break
    return out


# revision 15
# speedup vs baseline: 1.1089x; 1.1089x over previous
"""Trainium2 Bass kernel for nn_AffineAttentionNN (moe_routing).

Math (per the reference):
    dist_sq[n,c] = ||x[n]-ctrs[c]||^2_s   (s-weighted squared distance)
    a = softmax(-dist_sq, axis=c)
    out = einsum('nc,ng,cgp->np', a, x, Wv) + a @ Ov

Device decomposition (data-parallel over n across 8 cores; per core n_loc=2048):
  - Softmax offsets: the per-row term (x*x)@s is constant along c and cancels;
    we exponentiate g[c,n] = 2(x*s)@ctrs.T - ccs[c] directly.
  - PRE-normalized routing weights: aT = eT * (1/Z) in bf16, with 1/Z
    replicated to all partitions by a one-row PE matmul (no DMA transpose
    roundtrip). This removes the output normalize AND bounds a in (0,1] so
    the z stream fits fp8 range.
  - Per expert c: a_c must appear on all 128 partitions to scale x. Supply:
      * partition-broadcast DMA of PAIRS of adjacent a-rows (1 MB) on BOTH
        hardware DGE rings (SP/sync and Activation/scalar);
      * PE one-hot replication + ScalarE/DVE evacuation for the front pairs
        (hides the a->DRAM->broadcast latency) and a few tail pairs.
  - The multiply z = x * ar runs on the DVE (2 elem/cyc/lane bf16) - the
    critical engine at ~147us. GPSIMD cannot help (shared SBUF ports).
  - fp8 DoubleRow pairs: for FP8_PAIRS mid-kernel pairs, the otherwise-idle
    ScalarE converts z -> z8 = fp8e4(z * 2^-4) and ONE DoubleRow matmul per
    chunk computes both experts of the pair at 0.5 cyc/col against
    fp8(Wv * 2^4) - halving PE work (and power: the device HAM-throttles to
    50% util under sustained load) for those pairs. The 2^-4/2^4 scale split
    keeps the shared PSUM accumulation consistent with the bf16 pairs.
      gT = matmul(lhsT=2*s*ctrs (g,c), rhs=xT) (f32r)        [c, n] PSUM
      eT = Exp(gT + bias=-ccs[c])  on ScalarE -> bf16        [c, n]
      Z  = matmul(lhsT=ones, rhs=eT); rz=1/Z; rzrep via PE   [g, n] bf16
      aT = eT * rzrep (DVE)                                  [c, n] bf16
      per expert c: ar = broadcast(a[c,:]) via DMA or PE+ACT [g, n] bf16
                    z  = xT_bf * ar   (VectorE, bf16 2x)     [g, n]
                    bf16: outT[p, js] += matmul(Wv[c], z-half)
                    fp8:  z8 = ScalarE fp8(z/16);  outT += DoubleRow(
                          Wv8[pair] (g,2,p), z8 (g,2,n))     both experts
      outT += matmul(lhsT=Ov (c,p), rhs=aT)                  (Ov term)
      out  = copy(outT) -> DMA, host transposes.
"""

import os
import numpy as np
from contextlib import ExitStack

import concourse.bass as bass
import concourse.tile as tile
from concourse import mybir

N, D, C, P = 16384, 128, 128, 128
N_CORES = 8
N_LOC = N // N_CORES          # 2048
CHUNK = 512                   # PSUM bank width (fp32)
NCH = N_LOC // CHUNK          # 4

F32 = mybir.dt.float32
F32R = mybir.dt.float32r
BF16 = mybir.dt.bfloat16
FP8 = mybir.dt.float8e4
DR = mybir.MatmulPerfMode.DoubleRow

N_PAIRS = C // 2
# Leading pairs on the PE-replication path: their supply comes from aT in
# SBUF, hiding the a->DRAM->broadcast latency at kernel start.
FRONT_PE = int(os.environ.get("KERNEL_FRONT_PE", "6"))
# Trailing pairs on the PE path so the end of the broadcast stream can't
# starve the consumer.
TAIL_PE = int(os.environ.get("KERNEL_TAIL_PE", "2"))
MID_PE = int(os.environ.get("KERNEL_MID_PE", "0"))
PE_PAIRS = FRONT_PE + TAIL_PE + MID_PE
# How many experts ahead the ar production runs of the consuming multiply.
LOOKAHEAD = int(os.environ.get("KERNEL_LOOKAHEAD", "16"))
# Evacuate the FRONT pairs' first-half PSUM tiles on the (ramp-idle) DVE and
# the second halves on ScalarE, halving the startup load on each engine.
FRONT_DVE = os.environ.get("KERNEL_FRONT_DVE", "1") == "1"
# PE warmup: dummy matmuls at t=0 ramp the tensor engine's p-state.
WARMUP_MM = int(os.environ.get("KERNEL_WARMUP_MM", "6"))
# fp8 DoubleRow pairs: every other pair starting at FP8_FIRST, FP8_PAIRS of
# them. Alternation keeps the ScalarE conversion cadence (3.4us/pair) below
# the DVE pair cadence (2x 2.3us).
FP8_PAIRS = int(os.environ.get("KERNEL_FP8_PAIRS", "26"))
FP8_FIRST = int(os.environ.get("KERNEL_FP8_FIRST", "7"))
FP8_SCALE = 4  # z * 2^-4, Wv * 2^4


def _pair_is_pe(p):
    """FRONT_PE leading + TAIL_PE trailing pairs, Bresenham spread between."""
    if p < FRONT_PE or p >= N_PAIRS - TAIL_PE:
        return True
    rest = MID_PE
    span = N_PAIRS - FRONT_PE - TAIL_PE
    q = p - FRONT_PE
    return ((q + 1) * rest) // span - (q * rest) // span == 1


def _fp8_pairs():
    out = []
    p = FP8_FIRST
    while len(out) < FP8_PAIRS and p < N_PAIRS - TAIL_PE:
        if not _pair_is_pe(p):
            out.append(p)
        p += 2
    return out


_FP8_SET = set(_fp8_pairs())
_BF_PAIRS = [p for p in range(N_PAIRS) if p not in _FP8_SET]
_BF_SLOT = {p: i for i, p in enumerate(_BF_PAIRS)}
_FP8_SLOT = {p: i for i, p in enumerate(sorted(_FP8_SET))}


def _dedup_ldweights(nc):
    """Each matmul is emitted as an InstLdweights + non-self-loading
    InstMatmult pair; a run of matmuls sharing the same stationary operand
    re-loads it before every matmul, which blocks fill/drain overlap between
    them (379 ns/MM instead of ~216). Delete the redundant loads. The
    schedule is final here, so block order IS the PE execution order; the
    deleted loads carry no sync_info and no one references them. bf16/fp8
    only (f32r has a known walrus quirk around non-self-loading); the
    perf_mode is part of the signature so DoubleRow loads only dedup against
    identical DoubleRow loads."""
    n = 0
    for f in nc.m.functions:
        for blk in f.blocks:
            last_sig = None
            keep = []
            for inst in blk.instructions:
                if str(inst.engine) != "EngineType.PE":
                    keep.append(inst)
                    continue
                if isinstance(inst, mybir.InstLdweights):
                    w = inst.ins[0]
                    si = inst.sync_info
                    clean = si is None or (not si.on_wait and not si.on_update)
                    if (w.dtype in (mybir.dt.bfloat16, mybir.dt.float8e4)
                            and clean
                            and inst.is_transpose is None):
                        sig = (str(w.ap), w.offset, str(w.memref),
                               str(inst.perf_mode))
                        if sig == last_sig:
                            n += 1
                            continue  # drop the redundant load
                        last_sig = sig
                    else:
                        last_sig = None
                elif isinstance(inst, mybir.InstMatmult):
                    if inst.ldweights is not False or inst.is_transpose:
                        last_sig = None  # self-loading matmul replaces weights
                elif isinstance(inst, (mybir.InstEventSemaphore, mybir.InstDrain)):
                    pass  # no effect on the loaded weights
                else:
                    last_sig = None
                keep.append(inst)
            blk.instructions = keep
    return n


def _legalize_waits(nc, max_waits=1):
    """This walrus build accepts at most one sync-wait per instruction; Tile
    emits several. Hoist the excess onto standalone single-wait
    InstEventSemaphore ops just before the owner on the same engine stream."""
    import bass_rust

    n = 0
    for f in nc.m.functions:
        for blk in f.blocks:
            out = []
            for inst in blk.instructions:
                si = getattr(inst, "sync_info", None)
                waits = list(si.on_wait) if si is not None else []
                if len(waits) > max_waits:
                    extra, keep = waits[:-max_waits], waits[-max_waits:]
                    for w in extra:
                        n += 1
                        ev = mybir.InstEventSemaphore(
                            name=f"legal_wait_{n}_{inst.name}", ins=[], outs=[]
                        )
                        ev.engine = inst.engine
                        ev.sync_info = bass_rust.SyncInfo(on_wait=[w], on_update=[])
                        out.append(ev)
                    inst.sync_info = bass_rust.SyncInfo(
                        on_wait=keep, on_update=list(si.on_update)
                    )
                out.append(inst)
            blk.instructions = out
    return n


def _emit_kernel(tc, aps):
    nc = tc.nc
    xT, wvb, wv8, bfpack, c2sT, nccs, outT = (
        aps["xT"], aps["wvb"], aps["wv8"], aps["bfpack"], aps["c2sT"],
        aps["nccs"], aps["outT"],
    )

    with ExitStack() as ctx:
        const = ctx.enter_context(tc.tile_pool(name="const", bufs=1))
        dram = ctx.enter_context(tc.tile_pool(name="dram", bufs=1, space="DRAM"))
        erep_p = ctx.enter_context(tc.tile_pool(name="erep", bufs=8))
        erpe_p = ctx.enter_context(tc.tile_pool(name="erpe", bufs=3))
        z_p = ctx.enter_context(tc.tile_pool(name="zt", bufs=4))
        z8_p = ctx.enter_context(tc.tile_pool(name="z8", bufs=3))
        out_p = ctx.enter_context(tc.tile_pool(name="outs", bufs=1))

        # ---- PE warmup: no-dep dummy matmuls ramp the clock during the
        # prologue DMAs ----
        ones_s = const.tile([C, 1], BF16, tag="ones")
        nc.vector.memset(ones_s[:], 1.0)
        ones1_s = const.tile([1, C], BF16, tag="ones1")
        nc.vector.memset(ones1_s[:], 1.0)
        if WARMUP_MM:
            warm_in = const.tile([C, CHUNK], BF16, tag="warm_in")
            nc.vector.memset(warm_in[:], 0.0)
            with tc.tile_pool(name="psum_w", bufs=1, space="PSUM") as psum_w:
                pw = psum_w.tile([C, CHUNK], F32, tag="pw")
                sel_w = ones_s[:, 0:1].broadcast_to([C, C])
                for _ in range(WARMUP_MM):
                    nc.tensor.matmul(pw[:], sel_w, warm_in[:],
                                     start=True, stop=True)

        # ---- constants / inputs into SBUF ----
        # The prologue gates the whole pipeline: its inputs load first on the
        # sync ring, fewest triggers first. The bf16 constants (id | ov |
        # xTb) ride as ONE host-packed DMA. Wv rides the SWDGE ring (GPSIMD
        # is otherwise idle; both HWDGE rings are saturated by a-broadcasts).
        c2s_s = const.tile([D, C], F32R, tag="c2s")
        nc.sync.dma_start(c2s_s[:], c2sT[:, :])
        nccs_s = const.tile([C, 1], F32, tag="nccs")
        nc.sync.dma_start(nccs_s[:], nccs[:, :])
        # prewarm the exp table off the first const load so ACT_TABLE_LOAD
        # isn't serialized into the first real activation's dependency chain
        warm_s = const.tile([C, 1], F32, tag="warm")
        nc.scalar.activation(
            warm_s[:], nccs_s[:], mybir.ActivationFunctionType.Exp)
        xT_s = const.tile([D, N_LOC], F32R, tag="xT")
        nc.sync.dma_start(xT_s[:, 0:CHUNK], xT[:, 0:CHUNK])
        bfp_s = const.tile([C, C + P + N_LOC], BF16, tag="bfp")
        nc.sync.dma_start(bfp_s[:], bfpack[:, :])
        id_s = bfp_s[:, 0:C]
        ov_s = bfp_s[:, C:C + P]
        xTb_s = bfp_s[:, C + P:]
        for j in range(1, NCH):
            js = slice(j * CHUNK, (j + 1) * CHUNK)
            nc.sync.dma_start(xT_s[:, js], xT[:, js])
        NB, NF = len(_BF_PAIRS), len(_FP8_SET)
        wvb_s = const.tile([D, NB * 2 * P], BF16, tag="wvb")
        WVCH = (NB * 2 * P) // 4
        for k in range(4):
            nc.gpsimd.dma_start(
                wvb_s[:, k * WVCH:(k + 1) * WVCH], wvb[:, k * WVCH:(k + 1) * WVCH])
        wv8_s = const.tile([D, max(NF, 1) * 2 * P], FP8, tag="wv8")
        if NF:
            W8CH = (NF * 2 * P) // 2
            for k in range(2):
                nc.gpsimd.dma_start(
                    wv8_s[:, k * W8CH:(k + 1) * W8CH],
                    wv8[:, k * W8CH:(k + 1) * W8CH])

        eT_s = const.tile([C, N_LOC], BF16, tag="eT")
        aT_s = const.tile([C, N_LOC], BF16, tag="aT")
        rzc_s = const.tile([1, N_LOC], F32, tag="rzc")
        rzb_s = const.tile([1, N_LOC], BF16, tag="rzb")
        rzrep_s = const.tile([P, N_LOC], BF16, tag="rzrep")

        e_dram = dram.tile([N_PAIRS, 2 * N_LOC], BF16, tag="e_dram")

        # ---- prologue: distances -> pre-normalized softmax weights aT ----
        with ExitStack() as dctx:
            psum_d = dctx.enter_context(
                tc.tile_pool(name="psum_d", bufs=2, space="PSUM"))
            psum_z = dctx.enter_context(
                tc.tile_pool(name="psum_z", bufs=1, space="PSUM"))
            psum_zr = dctx.enter_context(
                tc.tile_pool(name="psum_zr", bufs=1, space="PSUM"))
            for j in range(NCH):
                js = slice(j * CHUNK, (j + 1) * CHUNK)
                pd = psum_d.tile([C, CHUNK], F32, tag="pd")
                nc.tensor.matmul(pd[:], c2s_s[:], xT_s[:, js], start=True, stop=True)
                nc.scalar.activation(
                    eT_s[:, js], pd[:], mybir.ActivationFunctionType.Exp,
                    bias=nccs_s[:, 0:1], scale=1.0,
                )
                pz = psum_z.tile([1, CHUNK], F32, tag="pz")
                nc.tensor.matmul(pz[:], ones_s[:], eT_s[:, js], start=True, stop=True)
                # 1/Z, then replicate to all 128 partitions via a one-row
                # matmul (K=1): rzrep[:, js] = ones1.T @ rzb[0, js]
                nc.vector.reciprocal(rzc_s[0:1, js], pz[0:1, :])
                nc.scalar.copy(rzb_s[0:1, js], rzc_s[0:1, js])
                pzr = psum_zr.tile([P, CHUNK], F32, tag="pzr")
                nc.tensor.matmul(pzr[:], ones1_s[:], rzb_s[0:1, js],
                                 start=True, stop=True)
                nc.scalar.copy(rzrep_s[:, js], pzr[:])
                nc.vector.tensor_mul(aT_s[:, js], eT_s[:, js], rzrep_s[:, js])

        # a -> DRAM (paired-row layout) for the partition-broadcast DMAs
        e_flat = e_dram[:, :].rearrange("a (b n) -> (a b) n", b=2)
        nc.sync.dma_start(e_flat, aT_s[:])

        # ---- main expert loop, accumulate outT in PSUM ----
        psum_o = ctx.enter_context(tc.tile_pool(name="psum_o", bufs=1, space="PSUM"))
        psum_r = ctx.enter_context(tc.tile_pool(name="psum_r", bufs=2, space="PSUM"))
        po = psum_o.tile([P, N_LOC], F32, tag="po")

        # Ov term opens each chunk's accumulation group (it only needs aT), so
        # the tail chain is just last-expert-matmul -> copy -> store.
        for j in range(NCH):
            js = slice(j * CHUNK, (j + 1) * CHUNK)
            nc.tensor.matmul(
                po[:, js], ov_s[:], aT_s[:, js],
                start=True, stop=False, skip_group_check=True,
            )

        er_pair = {}   # pair -> er2 tile [D, 2*N_LOC]
        REP_HALF = N_LOC // 2
        # x repeated twice along a stride-0 dim, for one multiply per pair
        x_rep2 = xTb_s.unsqueeze(1).broadcast_to([D, 2, N_LOC])

        def emit_replicate(c):
            """PE one-hot matmul: replicate aT row c to all partitions, in two
            [D, N_LOC/2] PSUM tiles, each evacuated to SBUF bf16."""
            p, half = divmod(c, 2)
            if half == 0:
                erpe = erpe_p.tile([D, 2 * N_LOC], BF16, tag="erpe")
                er_pair[p] = erpe
            er = er_pair[p]
            # one-hot weights: identity column c repeated via a stride-0 dim
            sel_c = id_s[:, c:c + 1].broadcast_to([C, P])
            for h in range(2):
                dst = slice(half * N_LOC + h * REP_HALF,
                            half * N_LOC + (h + 1) * REP_HALF)
                pr = psum_r.tile([D, REP_HALF], F32, tag="pr")
                for q in range(REP_HALF // CHUNK):
                    qs = slice(q * CHUNK, (q + 1) * CHUNK)
                    src = slice(h * REP_HALF + q * CHUNK,
                                h * REP_HALF + (q + 1) * CHUNK)
                    nc.tensor.matmul(
                        pr[:, qs], sel_c, aT_s[:, src], start=True, stop=True)
                if FRONT_DVE and p < FRONT_PE and h == 0:
                    nc.vector.tensor_copy(er[:, dst], pr[:])
                else:
                    nc.scalar.copy(er[:, dst], pr[:])

        bc_count = [0]

        def emit_broadcast_pair(p):
            """One 1 MB DMA: broadcast a-rows (2p, 2p+1) to 128 partitions.
            Alternate between the two hardware DGE rings (sync / scalar)."""
            er2 = erep_p.tile([D, 2 * N_LOC], BF16, tag="er2")
            ring = nc.sync if bc_count[0] % 2 == 0 else nc.scalar
            bc_count[0] += 1
            ring.dma_start(er2[:], e_dram[p:p + 1, :].partition_broadcast(D))
            er_pair[p] = er2

        # Producer schedule: front pairs, then everything else in pair order.
        sched = []
        for p in range(N_PAIRS):
            if _pair_is_pe(p):
                sched.append(("rep", 2 * p, True))
                sched.append(("rep", 2 * p + 1, True))
            else:
                sched.append(("bc", p, True))
        prod_i = 0

        def pump(c_needed):
            nonlocal prod_i
            covered = -1
            while prod_i < len(sched) and covered < min(c_needed + LOOKAHEAD, C - 1):
                kind, v, counts = sched[prod_i]
                if kind == "bc":
                    emit_broadcast_pair(v)
                    covered = 2 * v + 1
                else:
                    emit_replicate(v)
                    if counts:
                        covered = v if v % 2 == 1 else v - 1
                prod_i += 1

        for p in range(N_PAIRS):
            pump(2 * p + 1)
            er2 = er_pair.pop(p)
            z2 = z_p.tile([D, 2 * N_LOC], BF16, tag="z2")
            if p < FRONT_PE:
                # startup: one multiply per expert so the first matmuls don't
                # wait for the whole pair's replication
                nc.vector.tensor_mul(z2[:, 0:N_LOC], xTb_s, er2[:, 0:N_LOC])
                nc.vector.tensor_mul(z2[:, N_LOC:], xTb_s, er2[:, N_LOC:])
            else:
                nc.vector.tensor_mul(z2[:], x_rep2, er2[:])
            if p in _FP8_SET:
                # ScalarE converts the pair's z to fp8 (scaled 2^-4); one
                # DoubleRow matmul per chunk then covers BOTH experts.
                z8 = z8_p.tile([D, 2 * N_LOC], FP8, tag="z8")
                nc.scalar.activation(
                    z8[:], z2[:], mybir.ActivationFunctionType.Copy,
                    scale=float(2.0 ** -FP8_SCALE))
                z8_r = z8[:].rearrange("d (two nc n) -> d nc two n",
                                       two=2, nc=NCH)
                f = _FP8_SLOT[p]
                w8 = wv8_s[:, f * 2 * P:(f + 1) * 2 * P].rearrange(
                    "d (two p) -> d two p", two=2)
                for j in range(NCH):
                    ps = slice(j * CHUNK, (j + 1) * CHUNK)
                    nc.tensor.matmul(
                        po[:, ps], w8, z8_r[:, j],
                        start=False, stop=(p == N_PAIRS - 1),
                        perf_mode=DR, skip_group_check=True,
                    )
            else:
                b = _BF_SLOT[p]
                for half in range(2):
                    wv_c = wvb_s[:, (2 * b + half) * P:(2 * b + half + 1) * P]
                    for j in range(NCH):
                        js = slice(half * N_LOC + j * CHUNK,
                                   half * N_LOC + (j + 1) * CHUNK)
                        ps = slice(j * CHUNK, (j + 1) * CHUNK)
                        nc.tensor.matmul(
                            po[:, ps], wv_c, z2[:, js],
                            start=False,
                            stop=(p == N_PAIRS - 1 and half == 1),
                            skip_group_check=True,
                        )

        # ---- copy and store per chunk; chunk j only waits on the last
        # expert's chunk-j matmul, so the tail pipelines across PE/DVE/DMA ----
        out_s = out_p.tile([P, N_LOC], F32, tag="out")
        for j in range(NCH):
            js = slice(j * CHUNK, (j + 1) * CHUNK)
            nc.vector.tensor_copy(out_s[:, js], po[:, js])
            nc.sync.dma_start(outT[:, js], out_s[:, js])


def build_nc():
    nc = bass.Bass(target_bir_lowering=False, trn_type="TRN2")
    NB, NF = len(_BF_PAIRS), len(_FP8_SET)
    aps = {
        "xT": nc.dram_tensor("xT", [D, N_LOC], F32R, kind="ExternalInput").ap(),
        "wvb": nc.dram_tensor("wvb", [D, NB * 2 * P], BF16,
                              kind="ExternalInput").ap(),
        "wv8": nc.dram_tensor("wv8", [D, max(NF, 1) * 2 * P], FP8,
                              kind="ExternalInput").ap(),
        "bfpack": nc.dram_tensor(
            "bfpack", [C, C + P + N_LOC], BF16, kind="ExternalInput").ap(),
        "c2sT": nc.dram_tensor("c2sT", [D, C], F32R, kind="ExternalInput").ap(),
        "nccs": nc.dram_tensor("nccs", [C, 1], F32, kind="ExternalInput").ap(),
        "outT": nc.dram_tensor("outT", [P, N_LOC], F32, kind="ExternalOutput").ap(),
    }
    with tile.TileContext(nc) as tc:
        _emit_kernel(tc, aps)
    _dedup_ldweights(nc)
    _legalize_waits(nc)
    return nc


_CACHE = {}


def _get_nc():
    key = (PE_PAIRS, LOOKAHEAD, WARMUP_MM, FP8_PAIRS, FP8_FIRST)
    if key not in _CACHE:
        _CACHE[key] = build_nc()
    return _CACHE[key]


def kernel(x, ctrs, Wv, Ov, s, _spmd_kwargs=None):
    import ml_dtypes
    from concourse.bass_utils import run_bass_kernel_spmd

    bf16 = ml_dtypes.bfloat16
    e4m3 = ml_dtypes.float8_e4m3
    x = np.ascontiguousarray(x, dtype=np.float32)
    ctrs = np.asarray(ctrs, dtype=np.float32)
    Wv = np.asarray(Wv, dtype=np.float32)
    Ov = np.ascontiguousarray(Ov, dtype=np.float32)
    s = np.asarray(s, dtype=np.float32)

    # host-side prep of the small routing constants
    c2sT = np.ascontiguousarray((2.0 * ctrs * s[None, :]).T)        # [g, c]
    nccs = np.ascontiguousarray(-((ctrs * ctrs) @ s)[:, None])      # [c, 1]
    wvT = Wv.transpose(1, 0, 2)                                     # [g, c, p]
    bf_experts = [2 * p + h for p in _BF_PAIRS for h in range(2)]
    f8_experts = [2 * p + h for p in sorted(_FP8_SET) for h in range(2)]
    wvb = np.ascontiguousarray(
        wvT[:, bf_experts].reshape(D, -1).astype(bf16))
    if f8_experts:
        wv8 = np.ascontiguousarray(
            (wvT[:, f8_experts] * float(2.0 ** FP8_SCALE))
            .reshape(D, -1).astype(e4m3))
    else:
        wv8 = np.zeros((D, 2 * P), dtype=e4m3)
    ov_b = Ov.astype(bf16)
    selT = np.eye(C, dtype=np.float32).astype(bf16)

    in_maps = []
    for i in range(N_CORES):
        xi = x[i * N_LOC:(i + 1) * N_LOC]
        xiT = np.ascontiguousarray(xi.T)
        # one bf16 constants DMA: [ id | Ov | x^T ] along the free dim
        bfpack = np.ascontiguousarray(
            np.concatenate([selT, ov_b, xiT.astype(bf16)], axis=1))
        m = {
            "xT": xiT,
            "wvb": wvb,
            "wv8": wv8,
            "bfpack": bfpack,
            "c2sT": c2sT,
            "nccs": nccs,
        }
        in_maps.append(m)

    nc = _get_nc()
    for attempt in range(3):
        res = run_bass_kernel_spmd(
            nc, in_maps, core_ids=list(range(N_CORES)), **(_spmd_kwargs or {})
        )
        out = np.empty((N, P), dtype=np.float32)
        for i in range(N_CORES):
            out[i * N_LOC:(i + 1) * N_LOC] = res.results[i]["outT"].T
        kernel.last_result = res
        if np.isfinite(out).all():
            break
    return out
